# revision 9
# baseline (speedup 1.0000x reference)
"""Trainium2 Bass kernel for a pre-norm transformer block with dilated
windowed causal attention (B=2, L=2048, D=512, H=8, DIL=2, WIN=256,
HIDDEN=2048).

Sharding: 8 cores = batch(2) x sequence-chunk(4 x 512 tokens). Each core
receives its 512-token chunk plus a 256-token halo (keys/values only) and
computes the full block for its tokens; no collectives.

Token axis is STREAM-MAJOR (dilation parity streams separated, host
reorders): t = [s0 own 256 | s1 own 256 | s0 halo 128 | s1 halo 128].
All attention slices are contiguous; host un-permutes the output.

Projection matmuls (QKV / out-proj / FFN) run in fp8e4 DoubleRow (two
128-deep k-planes per instruction). Weights are scaled x64 into fp8
range host-side; the descale is folded into the PSUM evacuation.
Attention S/PV matmuls stay bf16; S matmuls for the two heads of a
head-pair are emitted interleaved so they run on disjoint PE row groups
concurrently. The softmax denominator rides the PV matmul as a
ones-row; it is copied to SBUF, broadcast across partitions with a tiny
bf16 matmul (value 1/16), then inverted with reciprocal_approx_fast.
oT carries a x16 scale for fp8 range; out-proj descales by 1/(64*16).
FFN2 accumulation is interleaved into the FFN1 gelu pipeline.
"""
import os
import sys

os.environ.setdefault("MYCRO_LOCAL_CACHE", "1")
if "/opt/trn_rl_repo" not in sys.path:
    sys.path.insert(0, "/opt/trn_rl_repo")

import numpy as np

B, L, D, H, HD = 2, 2048, 512, 8, 64
HIDDEN = 4 * D
P = 128
CH = 512            # own tokens per core
HALO = 256
T = CH + HALO       # 768
NCORES = 8
EPS = 1e-5
SQ = 256            # own queries per parity stream
SCALE = 1.0 / 8.0   # 1/sqrt(HD)
WS = 64.0           # host-side fp8 weight scale
OS = 16.0           # oT fp8 scale (1/OS folded into esel)

NT = T // P         # 6
NO = CH // P        # 4
ND = D // P         # 4
NHID = HIDDEN // P  # 16

_nc = None
LAST_EXEC_NS = None
LAST_RESULTS = None

# key-tile -> t-range base: blk = s*3 + kt
KB = {(0, 0): 512, (0, 1): 0, (0, 2): 128,
      (1, 0): 640, (1, 1): 256, (1, 2): 384}


def _body(ctx, tc, I, y):
    import concourse.bass as bass  # noqa: F401
    from concourse import mybir

    nc = tc.nc
    f32 = mybir.dt.float32
    bf16 = mybir.dt.bfloat16
    f8 = mybir.dt.float8e4
    AF = mybir.ActivationFunctionType
    OP = mybir.AluOpType
    DR = mybir.MatmulPerfMode.DoubleRow

    consts = ctx.enter_context(tc.tile_pool(name="consts", bufs=1))
    big = ctx.enter_context(tc.tile_pool(name="big", bufs=1))
    work = ctx.enter_context(tc.tile_pool(name="work", bufs=4))
    pexp = ctx.enter_context(tc.tile_pool(name="pexp", bufs=8))

    mm = nc.tensor.matmul

    def bcast(ap, p=P):
        return bass.AP(tensor=ap.tensor, offset=ap.offset,
                       ap=[[0, p]] + [list(d) for d in ap.ap])

    # ---------- input DMAs (x first: critical path; weights behind) ----
    from concourse.tile import add_dep_helper

    ident = consts.tile([P, P], bf16, tag="ident")
    nc.sync.dma_start(out=ident, in_=I["ident"])
    x_sb = big.tile([P, NT, D], f32, tag="x")
    xdma = None
    for c0 in range(NT):
        xdma = nc.sync.dma_start(out=x_sb[:, c0:c0 + 1, :],
                                 in_=I["xc"][:, c0:c0 + 1, :])
    masks_sb = consts.tile([P, 2, 2 * SQ], bf16, tag="masks")
    wd = nc.sync.dma_start(out=masks_sb, in_=I["masks"])
    add_dep_helper(wd.ins, xdma.ins, reason="stagger mask DMA behind x")
    bq_sb = consts.tile([P, 4], f32, tag="bq")
    nc.sync.dma_start(out=bq_sb, in_=I["bq"])
    bk_sb = consts.tile([P, 4], f32, tag="bk")
    nc.sync.dma_start(out=bk_sb, in_=I["bk"])
    b1_sb = consts.tile([P, NHID], f32, tag="b1")
    nc.sync.dma_start(out=b1_sb, in_=I["b1"])
    # weights share SDMA bandwidth with x if launched together; make them
    # wait for the last x chunk so LN1 starts ~10us earlier.
    wqkv_sb = big.tile([P, ND, 3 * D], f8, tag="wqkv")
    wd = nc.sync.dma_start(out=wqkv_sb, in_=I["wqkvT"])
    add_dep_helper(wd.ins, xdma.ins, reason="stagger weight DMA behind x")
    wo_sb = big.tile([P, ND, D], f8, tag="wo")
    wd = nc.sync.dma_start(out=wo_sb, in_=I["woT"])
    add_dep_helper(wd.ins, xdma.ins, reason="stagger weight DMA behind x")
    w1_sb = big.tile([P, ND, HIDDEN], f8, tag="w1")
    wd = nc.sync.dma_start(out=w1_sb, in_=I["w1T"])
    add_dep_helper(wd.ins, xdma.ins, reason="stagger weight DMA behind x")
    w2_sb = big.tile([P, NHID, D], f8, tag="w2")
    wd = nc.sync.dma_start(out=w2_sb, in_=I["w2T"])
    add_dep_helper(wd.ins, xdma.ins, reason="stagger weight DMA behind x")
    bo_sb = consts.tile([P, D], f32, tag="bo")
    wd = nc.gpsimd.dma_start(out=bo_sb, in_=bcast(I["bo"]))
    add_dep_helper(wd.ins, xdma.ins, reason="stagger bcast DMA behind x")
    b2_sb = consts.tile([P, D], f32, tag="b2")
    wd = nc.gpsimd.dma_start(out=b2_sb, in_=bcast(I["b2"]))
    add_dep_helper(wd.ins, xdma.ins, reason="stagger bcast DMA behind x")

    epst = consts.tile([P, 1], f32, tag="eps")
    nc.vector.memset(epst, EPS)
    esel = consts.tile([P, P], bf16, tag="esel")
    nc.gpsimd.memset(esel, 0.0)
    nc.gpsimd.memset(esel[0:1, 0:64], 1.0 / OS)
    nc.gpsimd.memset(esel[64:65, 64:128], 1.0 / OS)

    # ---------- LN helper (stats+apply on Vector, sqrt on Scalar) ------
    def emit_ln(src, dst):
        st = work.tile([P, 6], f32, tag="bnst")
        nc.vector.bn_stats(st, src)
        mv = work.tile([P, 2], f32, tag="bnmv")
        nc.vector.bn_aggr(mv, st)
        r = work.tile([P, 1], f32, tag="lnr")
        nc.scalar.activation(r, mv[:, 1:2], AF.Sqrt, bias=epst, scale=1.0)
        r2 = work.tile([P, 1], f32, tag="lnr2")
        nc.vector.reciprocal(r2, r)
        nc.vector.tensor_scalar(out=dst, in0=src, scalar1=mv[:, 0:1],
                                scalar2=r2, op0=OP.subtract, op1=OP.mult)

    xhat = big.tile([P, NT, D], bf16, tag="xhat")
    xT = big.tile([P, ND, T], f8, tag="xT")
    kT = big.tile([P, 4, T], bf16, tag="kT")
    qT = big.tile([P, 4, CH], bf16, tag="qT")
    v_sb = big.tile([P, 6, H, 65], bf16, tag="v")

    with tc.tile_pool(name="pmm_h", bufs=2, space="PSUM") as pmm_h, \
         tc.tile_pool(name="ptp_h", bufs=2, space="PSUM") as ptp_h:

        # PE warm-up: keep the HAM activity window busy
        junk = pmm_h.tile([P, 512], f32, tag="ps")
        for _ in range(10):
            mm(junk[:, :P], ident, ident, start=True, stop=True)

        def emit_tp(cp):
            for dt_ in range(ND):
                pt = ptp_h.tile([P, 2 * P], bf16, tag="pt")
                for jj in range(2):
                    nc.tensor.transpose(pt[:, jj * P:(jj + 1) * P],
                                        xhat[:, 2 * cp + jj, dt_ * P:(dt_ + 1) * P],
                                        ident)
                if dt_ % 2 == 0:
                    nc.scalar.copy(xT[:, dt_, cp * 2 * P:(cp + 1) * 2 * P], pt)
                else:
                    nc.vector.tensor_copy(xT[:, dt_, cp * 2 * P:(cp + 1) * 2 * P], pt)

        def emit_k(t0, tn):
            for ot in range(4):
                ps = pmm_h.tile([P, 512], f32, tag="ps")
                for dp in range(2):
                    mm(ps[:, :tn], wqkv_sb[:, 2 * dp:2 * dp + 2, (4 + ot) * P:(5 + ot) * P],
                       xT[:, 2 * dp:2 * dp + 2, t0:t0 + tn],
                       start=(dp == 0), stop=(dp == 1), perf_mode=DR)
                if ot % 2 == 0:
                    nc.scalar.activation(kT[:, ot, t0:t0 + tn], ps[:, :tn],
                                         AF.Identity, bias=bk_sb[:, ot:ot + 1],
                                         scale=1.0 / WS)
                else:
                    nc.vector.tensor_scalar(out=kT[:, ot, t0:t0 + tn],
                                            in0=ps[:, :tn], scalar1=1.0 / WS,
                                            scalar2=bk_sb[:, ot:ot + 1],
                                            op0=OP.mult, op1=OP.add)

        def emit_q():
            for ot in range(4):
                ps = pmm_h.tile([P, 512], f32, tag="ps")
                for dp in range(2):
                    mm(ps, wqkv_sb[:, 2 * dp:2 * dp + 2, ot * P:(ot + 1) * P],
                       xT[:, 2 * dp:2 * dp + 2, 0:CH],
                       start=(dp == 0), stop=(dp == 1), perf_mode=DR)
                nc.scalar.activation(qT[:, ot, :], ps, AF.Identity,
                                     bias=bq_sb[:, ot:ot + 1], scale=1.0 / WS)

        def emit_v(s, kt):
            t0 = KB[(s, kt)]
            blk = s * 3 + kt
            ps = pmm_h.tile([P, 512], f32, tag="ps")
            for dp in range(2):
                mm(ps, xT[:, 2 * dp:2 * dp + 2, t0:t0 + P],
                   wqkv_sb[:, 2 * dp:2 * dp + 2, 2 * D:3 * D],
                   start=(dp == 0), stop=(dp == 1), perf_mode=DR)
            nc.vector.tensor_scalar(out=v_sb[:, blk, :, 0:64],
                                    in0=ps.rearrange("p (h c) -> p h c", h=H),
                                    scalar1=1.0 / WS, scalar2=0.0,
                                    op0=OP.mult, op1=OP.add)

        nc.gpsimd.memset(v_sb[:, :, :, 64:65], 1.0)

        emit_ln(x_sb[:, 0, :], xhat[:, 0, :])
        emit_ln(x_sb[:, 1, :], xhat[:, 1, :])
        emit_tp(0)
        emit_ln(x_sb[:, 2, :], xhat[:, 2, :])
        emit_ln(x_sb[:, 3, :], xhat[:, 3, :])
        emit_tp(1)
        emit_q()
        emit_k(0, 512)
        emit_v(0, 1)
        emit_v(0, 2)
        emit_ln(x_sb[:, 4, :], xhat[:, 4, :])
        emit_ln(x_sb[:, 5, :], xhat[:, 5, :])
        emit_tp(2)
        emit_k(512, 256)
        emit_v(1, 1)
        emit_v(1, 2)
        emit_v(0, 0)
        emit_v(1, 0)

        # out-proj bias pre-add (after LN reads of x_sb; used by out-proj)
        for tt in range(NO):
            nc.gpsimd.tensor_add(x_sb[:, tt, :], x_sb[:, tt, :], bo_sb)

    # ---------- attention ----------
    # masks_sb[:, 0] = [tril|tril|triu|triu]  (kt0 s0,s1 | kt2 s0,s1)
    # masks_sb[:, 1] = [triu|tril|triu|tril]  (kt1: s0 qb0,qb1 | s1 qb0,qb1)
    oU = big.tile([P, 4, CH], bf16, tag="oU")
    oT = big.tile([P, 4, CH], f8, tag="oT")
    den4s = {}
    for hp in range(4):
        den = work.tile([P, CH], bf16, tag="den")
        den4s[hp] = den
        nc.gpsimd.memset(den, 1.0)

    with tc.tile_pool(name="pa_s", bufs=4, space="PSUM") as pa_s, \
         tc.tile_pool(name="pa_o", bufs=3, space="PSUM") as pa_o:

        def emit_S02(hp, alt):
            ps = {hh: pa_s.tile([P, 2 * SQ], f32, tag="ps_s", name=f"ps02_{hp}_{hh}")
                  for hh in (0, 1)}
            for hh in (0, 1):
                mm(ps[hh], ident, masks_sb[:, 0, :], start=True, stop=False)
            for ri, (s, kt) in enumerate([(0, 0), (1, 0), (0, 2), (1, 2)]):
                k0 = KB[(s, kt)]
                q0 = s * SQ if kt == 0 else s * SQ + P
                for hh in (0, 1):
                    lo = hh * 64
                    mm(ps[hh][:, ri * P:(ri + 1) * P],
                       kT[lo:lo + 64, hp, k0:k0 + P],
                       qT[lo:lo + 64, hp, q0:q0 + P],
                       start=False, stop=(ri == 3))
            out = {}
            for hh in (0, 1):
                p_sb = pexp.tile([P, 2 * SQ], bf16, tag="p_sb")
                nc.scalar.activation(p_sb, ps[hh], AF.Exp, scale=SCALE)
                out[hh] = p_sb
            return out

        def emit_S1(hp, alt):
            ps = {hh: pa_s.tile([P, 2 * SQ], f32, tag="ps_s", name=f"ps1_{hp}_{hh}")
                  for hh in (0, 1)}
            for hh in (0, 1):
                mm(ps[hh], ident, masks_sb[:, 1, :], start=True, stop=False)
            for s in (0, 1):
                k0 = KB[(s, 1)]
                q0 = s * SQ
                for hh in (0, 1):
                    lo = hh * 64
                    mm(ps[hh][:, s * SQ:(s + 1) * SQ],
                       kT[lo:lo + 64, hp, k0:k0 + P],
                       qT[lo:lo + 64, hp, q0:q0 + SQ],
                       start=False, stop=(s == 1))
            out = {}
            for hh in (0, 1):
                p_sb = pexp.tile([P, 2 * SQ], bf16, tag="p_sb")
                nc.scalar.activation(p_sb, ps[hh], AF.Exp, scale=SCALE)
                out[hh] = p_sb
            return out

        def emit_PV(hp, hh, p02, p1):
            h = 2 * hp + hh
            lo = hh * 64
            po = pa_o.tile([P, 2 * SQ], f32, tag="po")
            for s in range(2):
                qa = s * SQ
                qb = s * SQ + P
                mm(po[:65, qa:qa + P], v_sb[:, s * 3 + 0, h, :],
                   p02[:, s * P:(s + 1) * P], start=True, stop=False)
                mm(po[:65, qa:qa + P], v_sb[:, s * 3 + 1, h, :],
                   p1[:, s * 2 * P:s * 2 * P + P], start=False, stop=True)
                mm(po[:65, qb:qb + P], v_sb[:, s * 3 + 1, h, :],
                   p1[:, s * 2 * P + P:(s + 1) * 2 * P], start=True, stop=False)
                mm(po[:65, qb:qb + P], v_sb[:, s * 3 + 2, h, :],
                   p02[:, 2 * SQ // 2 + s * P:2 * SQ // 2 + (s + 1) * P],
                   start=False, stop=True)
            if hh == 0:
                nc.scalar.copy(oU[lo:lo + 64, hp, :], po[:64, :])
                with nc.allow_low_precision("softmax denominator in bf16"):
                    nc.vector.tensor_copy(den4s[hp][lo:lo + 1, :], po[64:65, :])
            else:
                nc.vector.tensor_copy(oU[lo:lo + 64, hp, :], po[:64, :])
                nc.scalar.copy(den4s[hp][lo:lo + 1, :], po[64:65, :])

        def emit_norm(hp):
            pb = pa_o.tile([P, 2 * SQ], f32, tag="po")
            mm(pb, esel, den4s[hp], start=True, stop=True)
            rb = work.tile([P, CH], f32, tag="rb")
            nc.vector.reciprocal_approx_fast(rb, pb)
            nc.vector.tensor_mul(oT[:, hp, :], oU[:, hp, :], rb)

        prev = None
        for hp in range(4):
            alt = hp % 2 == 0
            p02 = emit_S02(hp, alt)
            p1 = emit_S1(hp, alt)
            if prev is not None:
                php, p02p, p1p = prev
                emit_PV(php, 0, p02p[0], p1p[0])
                emit_PV(php, 1, p02p[1], p1p[1])
                emit_norm(php)
            prev = (hp, p02, p1)
        php, p02p, p1p = prev
        emit_PV(php, 0, p02p[0], p1p[0])
        emit_PV(php, 1, p02p[1], p1p[1])
        emit_norm(php)

    # ---------- tail: out-proj, LN2, x2T, FFN1+FFN2 interleaved --------
    res1 = big.tile([P, NO, D], f32, tag="res1")
    xhat2 = big.tile([P, NO, D], bf16, tag="xhat2")
    x2T = big.tile([P, ND, CH], f8, tag="x2T")
    g_sb = big.tile([P, NHID, CH], f8, tag="g")
    fin = big.tile([P, NO, D], f32, tag="fin")

    with tc.tile_pool(name="pmm_t", bufs=2, space="PSUM") as pmm_t, \
         tc.tile_pool(name="ptp_t", bufs=2, space="PSUM") as ptp_t, \
         tc.tile_pool(name="pffn", bufs=1, space="PSUM") as pffn:

        for tt in range(NO):
            ps = pmm_t.tile([P, 512], f32, tag="ps")
            for pp in range(2):
                mm(ps, oT[:, 2 * pp:2 * pp + 2, tt * P:(tt + 1) * P],
                   wo_sb[:, 2 * pp:2 * pp + 2, :],
                   start=(pp == 0), stop=(pp == 1), perf_mode=DR)
            nc.vector.scalar_tensor_tensor(out=res1[:, tt, :], in0=ps,
                                           scalar=1.0 / (WS * OS),
                                           in1=x_sb[:, tt, :],
                                           op0=OP.mult, op1=OP.add)
            emit_ln(res1[:, tt, :], xhat2[:, tt, :])
            if tt % 2 == 1:
                cp = tt // 2
                for dt_ in range(ND):
                    pt = ptp_t.tile([P, 2 * P], bf16, tag="pt")
                    for jj in range(2):
                        nc.tensor.transpose(pt[:, jj * P:(jj + 1) * P],
                                            xhat2[:, 2 * cp + jj, dt_ * P:(dt_ + 1) * P],
                                            ident)
                    if dt_ % 2 == 0:
                        nc.scalar.copy(x2T[:, dt_, cp * 2 * P:(cp + 1) * 2 * P], pt)
                    else:
                        nc.vector.tensor_copy(x2T[:, dt_, cp * 2 * P:(cp + 1) * 2 * P], pt)

        for tt in range(NO):
            nc.vector.tensor_add(res1[:, tt, :], res1[:, tt, :], b2_sb)

        ps_tt = [pffn.tile([P, 512], f32, tag=f"pf{tt}", name=f"pf{tt}") for tt in range(NO)]

        def emit_f2(hq):
            for tt in range(NO):
                mm(ps_tt[tt], g_sb[:, 2 * hq:2 * hq + 2, tt * P:(tt + 1) * P],
                   w2_sb[:, 2 * hq:2 * hq + 2, :],
                   start=(hq == 0), stop=(hq == NHID // 2 - 1), perf_mode=DR)

        for ht in range(NHID):
            ps = pmm_t.tile([P, 512], f32, tag="ps")
            for dp in range(2):
                mm(ps, w1_sb[:, 2 * dp:2 * dp + 2, ht * P:(ht + 1) * P],
                   x2T[:, 2 * dp:2 * dp + 2, :],
                   start=(dp == 0), stop=(dp == 1), perf_mode=DR)
            nc.scalar.activation(g_sb[:, ht, :], ps, AF.Gelu,
                                 bias=b1_sb[:, ht:ht + 1], scale=1.0 / WS)
            if ht >= 3 and ht % 2 == 1:
                emit_f2(ht // 2 - 1)
        emit_f2(NHID // 2 - 1)
        for tt in range(NO):
            nc.vector.scalar_tensor_tensor(out=fin[:, tt, :], in0=ps_tt[tt],
                                           scalar=1.0 / WS, in1=res1[:, tt, :],
                                           op0=OP.mult, op1=OP.add)

        yr = y.rearrange("(j p) d -> p j d", p=P)
        for tt in range(NO):
            eng = nc.sync if tt % 2 == 0 else nc.scalar
            eng.dma_start(out=yr[:, tt, :], in_=fin[:, tt, :])


def _build():
    from contextlib import ExitStack

    import concourse.bacc as bacc
    import concourse.tile as tile
    from concourse import mybir

    f32 = mybir.dt.float32
    bf16 = mybir.dt.bfloat16
    f8 = mybir.dt.float8e4
    nc = bacc.Bacc("TRN2", target_bir_lowering=False, debug=False,
                   enable_asserts=False, num_devices=NCORES)
    I = {}

    def inp(name, shape, dt_):
        I[name] = nc.dram_tensor(name, list(shape), dt_, kind="ExternalInput").ap()

    inp("xc", (P, NT, D), f32)
    inp("ident", (P, P), bf16)
    inp("wqkvT", (P, ND, 3 * D), f8)
    inp("bq", (P, 4), f32)
    inp("bk", (P, 4), f32)
    inp("woT", (P, ND, D), f8)
    inp("bo", (D,), f32)
    inp("w1T", (P, ND, HIDDEN), f8)
    inp("b1", (P, NHID), f32)
    inp("w2T", (P, NHID, D), f8)
    inp("b2", (D,), f32)
    inp("masks", (P, 2, 2 * SQ), bf16)
    y = nc.dram_tensor("y", [CH, D], f32, kind="ExternalOutput").ap()

    with tile.TileContext(nc) as tc:
        with ExitStack() as ctx:
            _body(ctx, tc, I, y)
    nc.compile()
    return nc


def _host_masks():
    """Additive pre-softmax masks: 0 where valid, -240 where masked
    (exp(-240/8) == 0 in bf16). Added into the S psum with an
    ident-lhsT matmul instead of multiplying after exp."""
    import ml_dtypes
    tril = np.tril(np.ones((P, P), np.float32))
    triu = np.triu(np.ones((P, P), np.float32))
    kt02 = np.concatenate([tril, tril, triu, triu], 1)
    kt1 = np.concatenate([triu, tril, triu, tril], 1)
    m = np.stack([kt02, kt1])
    m0 = m.copy()
    m0[0, :, 0:256] = 0.0  # first chunk of each batch: halo keys invalid
    m = ((m - 1.0) * 240.0).astype(ml_dtypes.bfloat16)
    m0 = ((m0 - 1.0) * 240.0).astype(ml_dtypes.bfloat16)
    m = np.ascontiguousarray(m.transpose(1, 0, 2))
    m0 = np.ascontiguousarray(m0.transpose(1, 0, 2))
    return m, m0


def get_nc():
    global _nc
    if _nc is None:
        _nc = _build()
    return _nc


def _pmaj(a, p=P):
    """[N*p, F...] row-major -> [p, N, F...] partition-major contiguous."""
    n = a.shape[0] // p
    return np.ascontiguousarray(
        a.reshape((n, p) + a.shape[1:]).transpose((1, 0) + tuple(range(2, a.ndim + 1))))


# stream-major chunk gather: full[768] = [halo 256 | own 512]
_a = np.arange(128)
_XIDX = np.concatenate([256 + 2 * _a, 512 + 2 * _a, 257 + 2 * _a,
                        513 + 2 * _a, 2 * _a, 1 + 2 * _a])
# y rows (chunk-major stream order) -> original own-token index
_TOKPERM = np.concatenate([2 * _a, 256 + 2 * _a, 1 + 2 * _a, 257 + 2 * _a])


def make_in_maps(inputs):
    import ml_dtypes
    f = np.float32
    bf = ml_dtypes.bfloat16
    f8 = ml_dtypes.float8_e4m3
    x = np.asarray(inputs["x"], f)
    qkv_w = np.asarray(inputs["qkv_w"], f)
    n1w = np.asarray(inputs["norm1_w"], f)
    n1b = np.asarray(inputs["norm1_b"], f)
    wqkv_f = qkv_w * n1w[None, :]
    bqkv = qkv_w @ n1b + np.asarray(inputs["qkv_b"], f)
    wqkvT = _pmaj(np.ascontiguousarray(wqkv_f.T) * WS).astype(f8)
    bq = np.ascontiguousarray(bqkv[0:D].reshape(4, P).T)
    bk = np.ascontiguousarray(bqkv[D:2 * D].reshape(4, P).T)
    bv = np.ascontiguousarray(bqkv[2 * D:3 * D])

    out_w = np.asarray(inputs["out_w"], f)
    woT = _pmaj(np.ascontiguousarray(out_w.T) * WS).astype(f8)
    bo = np.ascontiguousarray(out_w @ bv + np.asarray(inputs["out_b"], f))

    w1 = np.asarray(inputs["ffn_w1"], f)
    n2w = np.asarray(inputs["norm2_w"], f)
    n2b = np.asarray(inputs["norm2_b"], f)
    w1T = _pmaj(np.ascontiguousarray((w1 * n2w[None, :]).T) * WS).astype(f8)
    b1v = w1 @ n2b + np.asarray(inputs["ffn_b1"], f)
    b1 = np.ascontiguousarray(b1v.reshape(NHID, P).T)
    w2T = _pmaj(np.ascontiguousarray(np.asarray(inputs["ffn_w2"], f).T) * WS).astype(f8)
    b2 = np.ascontiguousarray(np.asarray(inputs["ffn_b2"], f))

    ident = np.eye(P, dtype=bf)
    masks, masks0 = _host_masks()
    shared = dict(ident=ident, wqkvT=wqkvT, bq=bq, bk=bk, woT=woT, bo=bo,
                  w1T=w1T, b1=b1, w2T=w2T, b2=b2)
    in_maps = []
    for c in range(NCORES):
        b_, i = divmod(c, 4)
        own = x[b_, i * CH:(i + 1) * CH]
        if i == 0:
            halo = np.zeros((HALO, D), f)
        else:
            halo = x[b_, i * CH - HALO:i * CH]
        full = np.concatenate([halo, own], 0)
        xc = np.ascontiguousarray(
            full[_XIDX].reshape(NT, P, D).transpose(1, 0, 2))
        in_maps.append(dict(xc=xc, masks=(masks if i > 0 else masks0), **shared))
    return in_maps


def kernel(**inputs):
    global LAST_EXEC_NS, LAST_RESULTS
    from concourse.bass_utils import run_bass_kernel_spmd

    nc = get_nc()
    in_maps = make_in_maps(inputs)
    trace = bool(int(os.environ.get("BASS_KERNEL_TRACE", "0")))
    res = run_bass_kernel_spmd(nc, in_maps, core_ids=list(range(NCORES)),
                               trace=trace)
    LAST_EXEC_NS = res.exec_time_ns
    LAST_RESULTS = res
    out = np.zeros((B, L, D), np.float32)
    for c, r in enumerate(res.results):
        b_, i = divmod(c, 4)
        out[b_, i * CH + _TOKPERM] = r["y"]
    return out


# revision 10
# speedup vs baseline: 1.2194x; 1.2194x over previous
"""Trainium2 Bass kernel for a pre-norm transformer block with dilated
windowed causal attention (B=2, L=2048, D=512, H=8, DIL=2, WIN=256,
HIDDEN=2048).

Sharding: 8 cores = batch(2) x sequence-chunk(4 x 512 tokens). Each core
receives its 512-token chunk plus a 256-token halo (keys/values only) and
computes the full block for its tokens; no collectives.

Token axis is STREAM-MAJOR (dilation parity streams separated, host
reorders): t = [s0 own 256 | s1 own 256 | s0 halo 128 | s1 halo 128].
All attention slices are contiguous; host un-permutes the output.

Projection matmuls (QKV / out-proj / FFN) run in fp8e4 DoubleRow (two
128-deep k-planes per instruction). Weights are scaled x64 into fp8
range host-side; the descale is folded into the PSUM evacuation.
Attention S/PV matmuls stay bf16; S matmuls for the two heads of a
head-pair are emitted interleaved so they run on disjoint PE row groups
concurrently. The softmax denominator rides the PV matmul as a
ones-row; it is copied to SBUF, broadcast across partitions with a tiny
bf16 matmul (value 1/16), then inverted with reciprocal_approx_fast.
oT carries a x16 scale for fp8 range; out-proj descales by 1/(64*16).
FFN2 accumulation is interleaved into the FFN1 gelu pipeline.
"""
import os
import sys

os.environ.setdefault("MYCRO_LOCAL_CACHE", "1")
if "/opt/trn_rl_repo" not in sys.path:
    sys.path.insert(0, "/opt/trn_rl_repo")

import numpy as np

B, L, D, H, HD = 2, 2048, 512, 8, 64
HIDDEN = 4 * D
P = 128
CH = 512            # own tokens per core
HALO = 256
T = CH + HALO       # 768
NCORES = 8
EPS = 1e-5
SQ = 256            # own queries per parity stream
SCALE = 1.0 / 8.0   # 1/sqrt(HD)
WS = 64.0           # host-side fp8 weight scale
OS = 16.0           # oT fp8 scale (1/OS folded into esel)

NT = T // P         # 6
NO = CH // P        # 4
ND = D // P         # 4
NHID = HIDDEN // P  # 16

_nc = None
LAST_EXEC_NS = None
LAST_RESULTS = None

# key-tile -> t-range base: blk = s*3 + kt
KB = {(0, 0): 512, (0, 1): 0, (0, 2): 128,
      (1, 0): 640, (1, 1): 256, (1, 2): 384}


def _body(ctx, tc, I, y):
    import concourse.bass as bass  # noqa: F401
    from concourse import mybir

    nc = tc.nc
    f32 = mybir.dt.float32
    bf16 = mybir.dt.bfloat16
    f8 = mybir.dt.float8e4
    AF = mybir.ActivationFunctionType
    OP = mybir.AluOpType
    DR = mybir.MatmulPerfMode.DoubleRow

    consts = ctx.enter_context(tc.tile_pool(name="consts", bufs=1))
    big = ctx.enter_context(tc.tile_pool(name="big", bufs=1))
    work = ctx.enter_context(tc.tile_pool(name="work", bufs=4))
    pexp = ctx.enter_context(tc.tile_pool(name="pexp", bufs=8))

    mm = nc.tensor.matmul

    def bcast(ap, p=P):
        return bass.AP(tensor=ap.tensor, offset=ap.offset,
                       ap=[[0, p]] + [list(d) for d in ap.ap])

    # ---------- input DMAs (x first: critical path; weights behind) ----
    from concourse.tile import add_dep_helper

    ident = consts.tile([P, P], bf16, tag="ident")
    nc.sync.dma_start(out=ident, in_=I["ident"])
    x_sb = big.tile([P, NT, D], f32, tag="x")
    xdma = None
    for c0 in range(0, NT, 2):
        xdma = nc.sync.dma_start(out=x_sb[:, c0:c0 + 2, :],
                                 in_=I["xc"][:, c0:c0 + 2, :])
    masks_sb = consts.tile([P, 2, 2 * SQ], bf16, tag="masks")
    wd = nc.sync.dma_start(out=masks_sb, in_=I["masks"])
    add_dep_helper(wd.ins, xdma.ins, reason="stagger mask DMA behind x")
    bq_sb = consts.tile([P, 4], f32, tag="bq")
    nc.sync.dma_start(out=bq_sb, in_=I["bq"])
    bk_sb = consts.tile([P, 4], f32, tag="bk")
    nc.sync.dma_start(out=bk_sb, in_=I["bk"])
    b1_sb = consts.tile([P, NHID], f32, tag="b1")
    nc.sync.dma_start(out=b1_sb, in_=I["b1"])
    # weights share SDMA bandwidth with x if launched together; make them
    # wait for the last x chunk so LN1 starts ~10us earlier.
    wqkv_sb = big.tile([P, ND, 3 * D], f8, tag="wqkv")
    wd = nc.sync.dma_start(out=wqkv_sb, in_=I["wqkvT"])
    add_dep_helper(wd.ins, xdma.ins, reason="stagger weight DMA behind x")
    wo_sb = big.tile([P, ND, D], f8, tag="wo")
    wd = nc.sync.dma_start(out=wo_sb, in_=I["woT"])
    add_dep_helper(wd.ins, xdma.ins, reason="stagger weight DMA behind x")
    w1_sb = big.tile([P, ND, HIDDEN], f8, tag="w1")
    wd = nc.sync.dma_start(out=w1_sb, in_=I["w1T"])
    add_dep_helper(wd.ins, xdma.ins, reason="stagger weight DMA behind x")
    w2_sb = big.tile([P, NHID, D], f8, tag="w2")
    wd = nc.sync.dma_start(out=w2_sb, in_=I["w2T"])
    add_dep_helper(wd.ins, xdma.ins, reason="stagger weight DMA behind x")
    bo_sb = consts.tile([P, D], f32, tag="bo")
    wd = nc.gpsimd.dma_start(out=bo_sb, in_=bcast(I["bo"]))
    add_dep_helper(wd.ins, xdma.ins, reason="stagger bcast DMA behind x")
    b2_sb = consts.tile([P, D], f32, tag="b2")
    wd = nc.gpsimd.dma_start(out=b2_sb, in_=bcast(I["b2"]))
    add_dep_helper(wd.ins, xdma.ins, reason="stagger bcast DMA behind x")

    epst = consts.tile([P, 1], f32, tag="eps")
    nc.vector.memset(epst, EPS)
    esel = consts.tile([P, P], bf16, tag="esel")
    nc.gpsimd.memset(esel, 0.0)
    nc.gpsimd.memset(esel[0:1, 0:64], 1.0 / OS)
    nc.gpsimd.memset(esel[64:65, 64:128], 1.0 / OS)

    # ---------- LN helper (stats+apply on Vector, sqrt on Scalar) ------
    def emit_ln(src, dst):
        st = work.tile([P, 6], f32, tag="bnst")
        nc.vector.bn_stats(st, src)
        mv = work.tile([P, 2], f32, tag="bnmv")
        nc.vector.bn_aggr(mv, st)
        r = work.tile([P, 1], f32, tag="lnr")
        nc.scalar.activation(r, mv[:, 1:2], AF.Sqrt, bias=epst, scale=1.0)
        r2 = work.tile([P, 1], f32, tag="lnr2")
        nc.vector.reciprocal(r2, r)
        nc.vector.tensor_scalar(out=dst, in0=src, scalar1=mv[:, 0:1],
                                scalar2=r2, op0=OP.subtract, op1=OP.mult)

    xhat = big.tile([P, NT, D], bf16, tag="xhat")
    xT = big.tile([P, ND, T], f8, tag="xT")
    kT = big.tile([P, 4, T], bf16, tag="kT")
    qT = big.tile([P, 4, CH], bf16, tag="qT")
    v_sb = big.tile([P, 6, H, 65], bf16, tag="v")

    with tc.tile_pool(name="pmm_h", bufs=2, space="PSUM") as pmm_h, \
         tc.tile_pool(name="ptp_h", bufs=2, space="PSUM") as ptp_h:

        # PE warm-up: keep the HAM activity window busy
        junk = pmm_h.tile([P, 512], f32, tag="ps")
        for _ in range(24):
            mm(junk[:, :P], ident, ident, start=True, stop=True)

        def emit_tp(cp):
            for dt_ in range(ND):
                pt = ptp_h.tile([P, 2 * P], bf16, tag="pt")
                for jj in range(2):
                    nc.tensor.transpose(pt[:, jj * P:(jj + 1) * P],
                                        xhat[:, 2 * cp + jj, dt_ * P:(dt_ + 1) * P],
                                        ident)
                if dt_ % 2 == 0:
                    nc.scalar.copy(xT[:, dt_, cp * 2 * P:(cp + 1) * 2 * P], pt)
                else:
                    nc.vector.tensor_copy(xT[:, dt_, cp * 2 * P:(cp + 1) * 2 * P], pt)

        def emit_k(t0, tn):
            for ot in range(4):
                ps = pmm_h.tile([P, 512], f32, tag="ps")
                for dp in range(2):
                    mm(ps[:, :tn], wqkv_sb[:, 2 * dp:2 * dp + 2, (4 + ot) * P:(5 + ot) * P],
                       xT[:, 2 * dp:2 * dp + 2, t0:t0 + tn],
                       start=(dp == 0), stop=(dp == 1), perf_mode=DR)
                if ot % 2 == 0:
                    nc.scalar.activation(kT[:, ot, t0:t0 + tn], ps[:, :tn],
                                         AF.Identity, bias=bk_sb[:, ot:ot + 1],
                                         scale=1.0 / WS)
                else:
                    nc.vector.tensor_scalar(out=kT[:, ot, t0:t0 + tn],
                                            in0=ps[:, :tn], scalar1=1.0 / WS,
                                            scalar2=bk_sb[:, ot:ot + 1],
                                            op0=OP.mult, op1=OP.add)

        def emit_q():
            for ot in range(4):
                ps = pmm_h.tile([P, 512], f32, tag="ps")
                for dp in range(2):
                    mm(ps, wqkv_sb[:, 2 * dp:2 * dp + 2, ot * P:(ot + 1) * P],
                       xT[:, 2 * dp:2 * dp + 2, 0:CH],
                       start=(dp == 0), stop=(dp == 1), perf_mode=DR)
                nc.scalar.activation(qT[:, ot, :], ps, AF.Identity,
                                     bias=bq_sb[:, ot:ot + 1], scale=1.0 / WS)

        def emit_v(s, kt):
            t0 = KB[(s, kt)]
            blk = s * 3 + kt
            ps = pmm_h.tile([P, 512], f32, tag="ps")
            for dp in range(2):
                mm(ps, xT[:, 2 * dp:2 * dp + 2, t0:t0 + P],
                   wqkv_sb[:, 2 * dp:2 * dp + 2, 2 * D:3 * D],
                   start=(dp == 0), stop=(dp == 1), perf_mode=DR)
            nc.vector.tensor_scalar(out=v_sb[:, blk, :, 0:64],
                                    in0=ps.rearrange("p (h c) -> p h c", h=H),
                                    scalar1=1.0 / WS, scalar2=0.0,
                                    op0=OP.mult, op1=OP.add)

        nc.gpsimd.memset(v_sb[:, :, :, 64:65], 1.0)

        emit_ln(x_sb[:, 0, :], xhat[:, 0, :])
        emit_ln(x_sb[:, 1, :], xhat[:, 1, :])
        emit_tp(0)
        emit_ln(x_sb[:, 2, :], xhat[:, 2, :])
        emit_ln(x_sb[:, 3, :], xhat[:, 3, :])
        emit_tp(1)
        emit_q()
        emit_k(0, 512)
        emit_v(0, 1)
        emit_v(0, 2)
        emit_ln(x_sb[:, 4, :], xhat[:, 4, :])
        emit_ln(x_sb[:, 5, :], xhat[:, 5, :])
        emit_tp(2)
        emit_k(512, 256)
        emit_v(1, 1)
        emit_v(1, 2)
        emit_v(0, 0)
        emit_v(1, 0)

        # out-proj bias pre-add (after LN reads of x_sb; used by out-proj)
        for tt in range(NO):
            nc.gpsimd.tensor_add(x_sb[:, tt, :], x_sb[:, tt, :], bo_sb)

    # ---------- attention ----------
    # masks_sb[:, 0] = [tril|tril|triu|triu]  (kt0 s0,s1 | kt2 s0,s1)
    # masks_sb[:, 1] = [triu|tril|triu|tril]  (kt1: s0 qb0,qb1 | s1 qb0,qb1)
    oU = big.tile([P, 4, CH], bf16, tag="oU")
    oT = big.tile([P, 4, CH], f8, tag="oT")
    den4s = {}
    for hp in range(4):
        den = work.tile([P, CH], bf16, tag="den")
        den4s[hp] = den
        nc.gpsimd.memset(den, 1.0)

    with tc.tile_pool(name="pa_s", bufs=4, space="PSUM") as pa_s, \
         tc.tile_pool(name="pa_o", bufs=3, space="PSUM") as pa_o:

        def emit_S02(hp, alt):
            ps = {hh: pa_s.tile([P, 2 * SQ], f32, tag="ps_s", name=f"ps02_{hp}_{hh}")
                  for hh in (0, 1)}
            for hh in (0, 1):
                mm(ps[hh], ident, masks_sb[:, 0, :], start=True, stop=False)
            for ri, (s, kt) in enumerate([(0, 0), (1, 0), (0, 2), (1, 2)]):
                k0 = KB[(s, kt)]
                q0 = s * SQ if kt == 0 else s * SQ + P
                for hh in (0, 1):
                    lo = hh * 64
                    mm(ps[hh][:, ri * P:(ri + 1) * P],
                       kT[lo:lo + 64, hp, k0:k0 + P],
                       qT[lo:lo + 64, hp, q0:q0 + P],
                       start=False, stop=(ri == 3))
            out = {}
            for hh in (0, 1):
                p_sb = pexp.tile([P, 2 * SQ], bf16, tag="p_sb")
                nc.scalar.activation(p_sb, ps[hh], AF.Exp, scale=SCALE)
                out[hh] = p_sb
            return out

        def emit_S1(hp, alt):
            ps = {hh: pa_s.tile([P, 2 * SQ], f32, tag="ps_s", name=f"ps1_{hp}_{hh}")
                  for hh in (0, 1)}
            for hh in (0, 1):
                mm(ps[hh], ident, masks_sb[:, 1, :], start=True, stop=False)
            for s in (0, 1):
                k0 = KB[(s, 1)]
                q0 = s * SQ
                for hh in (0, 1):
                    lo = hh * 64
                    mm(ps[hh][:, s * SQ:(s + 1) * SQ],
                       kT[lo:lo + 64, hp, k0:k0 + P],
                       qT[lo:lo + 64, hp, q0:q0 + SQ],
                       start=False, stop=(s == 1))
            out = {}
            for hh in (0, 1):
                p_sb = pexp.tile([P, 2 * SQ], bf16, tag="p_sb")
                nc.scalar.activation(p_sb, ps[hh], AF.Exp, scale=SCALE)
                out[hh] = p_sb
            return out

        def emit_PV(hp, hh, p02, p1):
            h = 2 * hp + hh
            lo = hh * 64
            po = pa_o.tile([P, 2 * SQ], f32, tag="po")
            for s in range(2):
                qa = s * SQ
                qb = s * SQ + P
                mm(po[:65, qa:qa + P], v_sb[:, s * 3 + 0, h, :],
                   p02[:, s * P:(s + 1) * P], start=True, stop=False)
                mm(po[:65, qa:qa + P], v_sb[:, s * 3 + 1, h, :],
                   p1[:, s * 2 * P:s * 2 * P + P], start=False, stop=True)
                mm(po[:65, qb:qb + P], v_sb[:, s * 3 + 1, h, :],
                   p1[:, s * 2 * P + P:(s + 1) * 2 * P], start=True, stop=False)
                mm(po[:65, qb:qb + P], v_sb[:, s * 3 + 2, h, :],
                   p02[:, 2 * SQ // 2 + s * P:2 * SQ // 2 + (s + 1) * P],
                   start=False, stop=True)
            if hh == 0:
                nc.scalar.copy(oU[lo:lo + 64, hp, :], po[:64, :])
                with nc.allow_low_precision("softmax denominator in bf16"):
                    nc.vector.tensor_copy(den4s[hp][lo:lo + 1, :], po[64:65, :])
            else:
                nc.vector.tensor_copy(oU[lo:lo + 64, hp, :], po[:64, :])
                nc.scalar.copy(den4s[hp][lo:lo + 1, :], po[64:65, :])

        def emit_norm(hp):
            pb = pa_o.tile([P, 2 * SQ], f32, tag="po")
            mm(pb, esel, den4s[hp], start=True, stop=True)
            rb = work.tile([P, CH], f32, tag="rb")
            nc.vector.reciprocal_approx_fast(rb, pb)
            nc.vector.tensor_mul(oT[:, hp, :], oU[:, hp, :], rb)

        prev = None
        for hp in range(4):
            alt = hp % 2 == 0
            p02 = emit_S02(hp, alt)
            p1 = emit_S1(hp, alt)
            if prev is not None:
                php, p02p, p1p = prev
                emit_PV(php, 0, p02p[0], p1p[0])
                emit_PV(php, 1, p02p[1], p1p[1])
                emit_norm(php)
            prev = (hp, p02, p1)
        php, p02p, p1p = prev
        emit_PV(php, 0, p02p[0], p1p[0])
        emit_PV(php, 1, p02p[1], p1p[1])
        emit_norm(php)

    # ---------- tail: out-proj, LN2, x2T, FFN1+FFN2 interleaved --------
    res1 = big.tile([P, NO, D], f32, tag="res1")
    xhat2 = big.tile([P, NO, D], bf16, tag="xhat2")
    x2T = big.tile([P, ND, CH], f8, tag="x2T")
    g_sb = big.tile([P, NHID, CH], f8, tag="g")
    fin = big.tile([P, NO, D], f32, tag="fin")

    with tc.tile_pool(name="pmm_t", bufs=2, space="PSUM") as pmm_t, \
         tc.tile_pool(name="ptp_t", bufs=2, space="PSUM") as ptp_t, \
         tc.tile_pool(name="pffn", bufs=1, space="PSUM") as pffn:

        for tt in range(NO):
            ps = pmm_t.tile([P, 512], f32, tag="ps")
            for pp in range(2):
                mm(ps, oT[:, 2 * pp:2 * pp + 2, tt * P:(tt + 1) * P],
                   wo_sb[:, 2 * pp:2 * pp + 2, :],
                   start=(pp == 0), stop=(pp == 1), perf_mode=DR)
            nc.vector.scalar_tensor_tensor(out=res1[:, tt, :], in0=ps,
                                           scalar=1.0 / (WS * OS),
                                           in1=x_sb[:, tt, :],
                                           op0=OP.mult, op1=OP.add)
            emit_ln(res1[:, tt, :], xhat2[:, tt, :])
            if tt % 2 == 1:
                cp = tt // 2
                for dt_ in range(ND):
                    pt = ptp_t.tile([P, 2 * P], bf16, tag="pt")
                    for jj in range(2):
                        nc.tensor.transpose(pt[:, jj * P:(jj + 1) * P],
                                            xhat2[:, 2 * cp + jj, dt_ * P:(dt_ + 1) * P],
                                            ident)
                    if dt_ % 2 == 0:
                        nc.scalar.copy(x2T[:, dt_, cp * 2 * P:(cp + 1) * 2 * P], pt)
                    else:
                        nc.vector.tensor_copy(x2T[:, dt_, cp * 2 * P:(cp + 1) * 2 * P], pt)

        for tt in range(NO):
            nc.vector.tensor_add(res1[:, tt, :], res1[:, tt, :], b2_sb)

        ps_tt = [pffn.tile([P, 512], f32, tag=f"pf{tt}", name=f"pf{tt}") for tt in range(NO)]

        def emit_f2(hq):
            for tt in range(NO):
                mm(ps_tt[tt], g_sb[:, 2 * hq:2 * hq + 2, tt * P:(tt + 1) * P],
                   w2_sb[:, 2 * hq:2 * hq + 2, :],
                   start=(hq == 0), stop=(hq == NHID // 2 - 1), perf_mode=DR)

        for ht in range(NHID):
            ps = pmm_t.tile([P, 512], f32, tag="ps")
            for dp in range(2):
                mm(ps, w1_sb[:, 2 * dp:2 * dp + 2, ht * P:(ht + 1) * P],
                   x2T[:, 2 * dp:2 * dp + 2, :],
                   start=(dp == 0), stop=(dp == 1), perf_mode=DR)
            nc.scalar.activation(g_sb[:, ht, :], ps, AF.Gelu,
                                 bias=b1_sb[:, ht:ht + 1], scale=1.0 / WS)
            if ht >= 3 and ht % 2 == 1:
                emit_f2(ht // 2 - 1)
        emit_f2(NHID // 2 - 1)
        for tt in range(NO):
            nc.vector.scalar_tensor_tensor(out=fin[:, tt, :], in0=ps_tt[tt],
                                           scalar=1.0 / WS, in1=res1[:, tt, :],
                                           op0=OP.mult, op1=OP.add)

        yr = y.rearrange("(j p) d -> p j d", p=P)
        for tt in range(NO):
            eng = nc.sync if tt % 2 == 0 else nc.scalar
            eng.dma_start(out=yr[:, tt, :], in_=fin[:, tt, :])


def _build():
    from contextlib import ExitStack

    import concourse.bacc as bacc
    import concourse.tile as tile
    from concourse import mybir

    f32 = mybir.dt.float32
    bf16 = mybir.dt.bfloat16
    f8 = mybir.dt.float8e4
    nc = bacc.Bacc("TRN2", target_bir_lowering=False, debug=False,
                   enable_asserts=False, num_devices=NCORES)
    I = {}

    def inp(name, shape, dt_):
        I[name] = nc.dram_tensor(name, list(shape), dt_, kind="ExternalInput").ap()

    inp("xc", (P, NT, D), f32)
    inp("ident", (P, P), bf16)
    inp("wqkvT", (P, ND, 3 * D), f8)
    inp("bq", (P, 4), f32)
    inp("bk", (P, 4), f32)
    inp("woT", (P, ND, D), f8)
    inp("bo", (D,), f32)
    inp("w1T", (P, ND, HIDDEN), f8)
    inp("b1", (P, NHID), f32)
    inp("w2T", (P, NHID, D), f8)
    inp("b2", (D,), f32)
    inp("masks", (P, 2, 2 * SQ), bf16)
    y = nc.dram_tensor("y", [CH, D], f32, kind="ExternalOutput").ap()

    with tile.TileContext(nc) as tc:
        with ExitStack() as ctx:
            _body(ctx, tc, I, y)
    nc.compile()
    return nc


def _host_masks():
    """Additive pre-softmax masks: 0 where valid, -240 where masked
    (exp(-240/8) == 0 in bf16). Added into the S psum with an
    ident-lhsT matmul instead of multiplying after exp."""
    import ml_dtypes
    tril = np.tril(np.ones((P, P), np.float32))
    triu = np.triu(np.ones((P, P), np.float32))
    kt02 = np.concatenate([tril, tril, triu, triu], 1)
    kt1 = np.concatenate([triu, tril, triu, tril], 1)
    m = np.stack([kt02, kt1])
    m0 = m.copy()
    m0[0, :, 0:256] = 0.0  # first chunk of each batch: halo keys invalid
    m = ((m - 1.0) * 240.0).astype(ml_dtypes.bfloat16)
    m0 = ((m0 - 1.0) * 240.0).astype(ml_dtypes.bfloat16)
    m = np.ascontiguousarray(m.transpose(1, 0, 2))
    m0 = np.ascontiguousarray(m0.transpose(1, 0, 2))
    return m, m0


def get_nc():
    global _nc
    if _nc is None:
        _nc = _build()
    return _nc


def _pmaj(a, p=P):
    """[N*p, F...] row-major -> [p, N, F...] partition-major contiguous."""
    n = a.shape[0] // p
    return np.ascontiguousarray(
        a.reshape((n, p) + a.shape[1:]).transpose((1, 0) + tuple(range(2, a.ndim + 1))))


# stream-major chunk gather: full[768] = [halo 256 | own 512]
_a = np.arange(128)
_XIDX = np.concatenate([256 + 2 * _a, 512 + 2 * _a, 257 + 2 * _a,
                        513 + 2 * _a, 2 * _a, 1 + 2 * _a])
# y rows (chunk-major stream order) -> original own-token index
_TOKPERM = np.concatenate([2 * _a, 256 + 2 * _a, 1 + 2 * _a, 257 + 2 * _a])


def make_in_maps(inputs):
    import ml_dtypes
    f = np.float32
    bf = ml_dtypes.bfloat16
    f8 = ml_dtypes.float8_e4m3
    x = np.asarray(inputs["x"], f)
    qkv_w = np.asarray(inputs["qkv_w"], f)
    n1w = np.asarray(inputs["norm1_w"], f)
    n1b = np.asarray(inputs["norm1_b"], f)
    wqkv_f = qkv_w * n1w[None, :]
    bqkv = qkv_w @ n1b + np.asarray(inputs["qkv_b"], f)
    wqkvT = _pmaj(np.ascontiguousarray(wqkv_f.T) * WS).astype(f8)
    bq = np.ascontiguousarray(bqkv[0:D].reshape(4, P).T)
    bk = np.ascontiguousarray(bqkv[D:2 * D].reshape(4, P).T)
    bv = np.ascontiguousarray(bqkv[2 * D:3 * D])

    out_w = np.asarray(inputs["out_w"], f)
    woT = _pmaj(np.ascontiguousarray(out_w.T) * WS).astype(f8)
    bo = np.ascontiguousarray(out_w @ bv + np.asarray(inputs["out_b"], f))

    w1 = np.asarray(inputs["ffn_w1"], f)
    n2w = np.asarray(inputs["norm2_w"], f)
    n2b = np.asarray(inputs["norm2_b"], f)
    w1T = _pmaj(np.ascontiguousarray((w1 * n2w[None, :]).T) * WS).astype(f8)
    b1v = w1 @ n2b + np.asarray(inputs["ffn_b1"], f)
    b1 = np.ascontiguousarray(b1v.reshape(NHID, P).T)
    w2T = _pmaj(np.ascontiguousarray(np.asarray(inputs["ffn_w2"], f).T) * WS).astype(f8)
    b2 = np.ascontiguousarray(np.asarray(inputs["ffn_b2"], f))

    ident = np.eye(P, dtype=bf)
    masks, masks0 = _host_masks()
    shared = dict(ident=ident, wqkvT=wqkvT, bq=bq, bk=bk, woT=woT, bo=bo,
                  w1T=w1T, b1=b1, w2T=w2T, b2=b2)
    in_maps = []
    for c in range(NCORES):
        b_, i = divmod(c, 4)
        own = x[b_, i * CH:(i + 1) * CH]
        if i == 0:
            halo = np.zeros((HALO, D), f)
        else:
            halo = x[b_, i * CH - HALO:i * CH]
        full = np.concatenate([halo, own], 0)
        xc = np.ascontiguousarray(
            full[_XIDX].reshape(NT, P, D).transpose(1, 0, 2))
        in_maps.append(dict(xc=xc, masks=(masks if i > 0 else masks0), **shared))
    return in_maps


def kernel(**inputs):
    global LAST_EXEC_NS, LAST_RESULTS
    from concourse.bass_utils import run_bass_kernel_spmd

    nc = get_nc()
    in_maps = make_in_maps(inputs)
    trace = bool(int(os.environ.get("BASS_KERNEL_TRACE", "0")))
    res = run_bass_kernel_spmd(nc, in_maps, core_ids=list(range(NCORES)),
                               trace=trace)
    LAST_EXEC_NS = res.exec_time_ns
    LAST_RESULTS = res
    out = np.zeros((B, L, D), np.float32)
    for c, r in enumerate(res.results):
        b_, i = divmod(c, 4)
        out[b_, i * CH + _TOKPERM] = r["y"]
    return out
